# revision 1
# baseline (speedup 1.0000x reference)
"""Trainium2 Bass kernel for nn_Blurring_Model: 3D Gaussian blur (9^3 PSF)
on x[8,1,128,128,128] fp32, data-parallel over batch across 8 NeuronCores.

Method (per core, volume V[128,128,128]):
  The 3D PSF is separable: three 9-tap 1D convolutions along D, H, W.
  Each 1D conv along the SBUF partition axis is a matmul with a banded
  128x128 matrix B (B[d, d'] = g[d - d' + 4]); zero band entries handle
  the 'same' zero padding exactly.

  Every pass uses the volume chunk as the matmul's stationary operand
  (lhsT) and streams B, so out = chunk^T @ B both convolves the current
  partition axis AND rotates the next axis onto partitions ("rotation
  matmul") - no explicit transposes:

    V0 [D,(Hmaj,Wmin)] --pass1 conv D (chunks: fixed w)--> V1 [H,(Dmaj,Wmin)]
    V1                 --pass2 conv H (chunks: fixed d)--> V2 [W,(Hmaj,Dmin)]
    V2                 --pass3 conv W (chunks: fixed h)--> V3 [D,(Hmaj,Wmin)]

  128 matmuls of [K=128, M=128] x [K=128, N] per pass.  In "f32r" mode the
  streamed B is padded to N=256 so the PE runs float32r at 1 cycle/row.
  PSUM->SBUF copies are batched 8 chunks at a time and alternate between
  the Vector and Scalar engines.
"""

import sys

if "/opt/trn_rl_repo" not in sys.path:
    sys.path.insert(0, "/opt/trn_rl_repo")

import numpy as np

KERNEL_SIZE = 9
SPACING = (4.0, 4.0, 4.0)
CENTER = (KERNEL_SIZE - 1) / 2.0
PAD = (KERNEL_SIZE - 1) // 2
P = 128
HW = P * P
N_CORES = 8

# "f32r" | "f32" | "fp16" | "bf16"
MODE = "f32r"

GRP = 8           # matmul chunks per PSUM group / copy
NGRP = P // GRP   # groups per pass

_cache = {}


def _gauss1d(sigma, spacing):
    s = float(sigma) / spacing
    xs = np.arange(KERNEL_SIZE, dtype=np.float64)
    g = np.exp(-((xs - CENTER) ** 2) / (2.0 * s * s))
    g = g / g.sum()
    return g.astype(np.float32)


def _banded(g, ncols):
    # B[d, d'] = g[d - d' + PAD] for |d - d'| <= PAD, else 0.
    B = np.zeros((P, ncols), dtype=np.float32)
    d = np.arange(P)
    for i in range(KERNEL_SIZE):
        off = i - PAD
        dp = d - off
        m = (dp >= 0) & (dp < P)
        B[d[m], dp[m]] = g[i]
    return B


def _mode_params(mode):
    import concourse.mybir as mybir

    f32 = mybir.dt.float32
    if mode == "fp16":
        return mybir.dt.float16, np.float16, 128, 96
    if mode == "bf16":
        import ml_dtypes

        return mybir.dt.bfloat16, np.dtype(ml_dtypes.bfloat16), 128, 96
    if mode == "f32":
        return f32, np.float32, 128, 96
    if mode == "f32r":
        return f32, np.float32, 256, 120
    raise ValueError(mode)


def _build(mode):
    """Builds the SPMD Bass module (single program, run on 8 cores)."""
    if mode in _cache:
        return _cache[mode]

    from contextlib import ExitStack

    import concourse.bacc as bacc
    import concourse.bass as bass
    import concourse.mybir as mybir
    import concourse.tile as tile

    f32 = mybir.dt.float32
    dt_dat, _, NB, n_warm = _mode_params(mode)
    f32r_mode = mode == "f32r"
    # dtype the matmuls consume (and the volume tiles are stored as)
    dt_vol = mybir.dt.float32r if f32r_mode else dt_dat

    nc = bacc.Bacc(trn_type="TRN2", target_bir_lowering=False, debug=False)
    x_in = nc.declare_dram_parameter(
        "x", [P, HW], dt_vol if f32r_mode else dt_dat, isOutput=False
    ).ap()
    b_in = nc.declare_dram_parameter("bmats", [P, 3 * NB], dt_dat, isOutput=False).ap()
    y_out = nc.declare_dram_parameter("y", [P, HW], f32, isOutput=True).ap()

    with ExitStack() as ctx:
        tc = ctx.enter_context(tile.TileContext(nc))
        vol = ctx.enter_context(tc.tile_pool(name="vol", bufs=3))
        consts = ctx.enter_context(tc.tile_pool(name="consts", bufs=1))
        pspool = ctx.enter_context(tc.tile_pool(name="ps", bufs=2, space="PSUM"))

        braw = consts.tile([P, 3 * NB], dt_dat, name="braw", tag="braw")
        nc.sync.dma_start(out=braw[:], in_=b_in[:])
        btile = consts.tile([P, 3 * NB], dt_vol, name="btile", tag="b")
        # engine copy rounds f32 -> f32r as the BIR verifier requires
        nc.vector.tensor_copy(out=btile[:], in_=braw[:])
        scratch = consts.tile([P, 128], f32, name="scratch", tag="scratch")

        v0 = vol.tile([P, HW], dt_vol, name="v0", tag="vol")
        # split across both HWDGE rings (SP + ACT) for DMA parallelism
        nc.sync.dma_start(out=v0[:, 0 : HW // 2], in_=x_in[:, 0 : HW // 2])
        nc.scalar.dma_start(out=v0[:, HW // 2 :], in_=x_in[:, HW // 2 :])

        # Two persistent PSUM tiles, ping-ponged by group parity.  Never
        # recycling tiles keeps the PE->PE PSUM WAW deps semaphore-free
        # (program order), so each matmul carries at most ONE sync wait -
        # the LDWEIGHTS instruction encoding cannot hold more.
        psA = pspool.tile([P, GRP * NB], f32, name="psA", tag="ps")
        psB = pspool.tile([P, GRP * NB], f32, name="psB", tag="ps")

        # Warm the ACT tables (Copy) and the PE HAM clock gate while the
        # 8MB input DMA is in flight.
        nc.scalar.copy(out=scratch[:], in_=braw[:, 0:128])
        for _ in range(n_warm):
            nc.tensor.matmul(
                out=psA[:, 0:NB],
                lhsT=btile[:, 0:128],
                rhs=btile[:, 0:NB],
                start=True,
                stop=True,
            )

        v1 = vol.tile([P, HW], dt_vol, name="v1", tag="vol")
        v2 = vol.tile([P, HW], dt_vol, name="v2", tag="vol")
        v3 = vol.tile([P, HW], f32, name="v3", tag="vol")

        def conv_pass(src, dst, b_idx, chunk_fn, dst_ap_fn, pass_idx):
            b_ap = btile[:, b_idx * NB : (b_idx + 1) * NB]
            dve_copies = []
            for g in range(NGRP):
                ps = psA if g % 2 == 0 else psB
                for c in range(GRP):
                    nc.tensor.matmul(
                        out=ps[:, c * NB : (c + 1) * NB],
                        lhsT=chunk_fn(src, g * GRP + c),
                        rhs=b_ap,
                        start=True,
                        stop=True,
                    )
                src_ap, dst_ap = dst_ap_fn(ps, dst, g)
                if g % 2 == 0:
                    cp = nc.vector.tensor_copy(out=dst_ap, in_=src_ap)
                    dve_copies.append(cp)
                else:
                    nc.scalar.copy(out=dst_ap, in_=src_ap)
                if pass_idx == 2:
                    dma_eng = nc.sync if g % 2 == 0 else nc.scalar
                    dma_eng.dma_start(
                        out=y_out[:, g * 1024 : (g + 1) * 1024],
                        in_=v3[:, g * 1024 : (g + 1) * 1024],
                    )
            return dve_copies

        # All passes: chunk = strided lhsT read (fixed minor index of the
        # free dim), copy dst = CONTIGUOUS 1024-elem block (so Tile's
        # subtile dep tracking sees disjoint copy writes - no spurious
        # cross-engine WAW waits).
        #
        # pass 1: conv D.  V0[d, h*128+w]; chunk w -> [d, h];
        # out [h, d'] -> V1[h, w*128 + d']
        def chunk1(src, w):
            return src.rearrange("p (h w) -> p w h", w=P)[:, w, :]

        # pass 2: conv H.  V1[h, w*128+d]; chunk d -> [h, w];
        # out [w, h'] -> V2[w, d*128 + h']
        def chunk2(src, d):
            return src.rearrange("p (w d) -> p d w", d=P)[:, d, :]

        # pass 3: conv W.  V2[w, d*128+h]; chunk h -> [w, d];
        # out [d, w'] -> V3[d, h*128 + w']
        def chunk3(src, h):
            return src.rearrange("p (d h) -> p h d", h=P)[:, h, :]

        def dst_block(ps, dst, g):
            src_ap = ps.rearrange("p (c n) -> p c n", n=NB)[:, :, 0:128]
            dst_ap = dst.rearrange("p (c n) -> p c n", n=128)[:, g * GRP : (g + 1) * GRP, :]
            return src_ap, dst_ap

        dst1 = dst2 = dst3 = dst_block

        from concourse.tile_rust import add_dep_helper

        def pass_boundary(dve_copies, idx):
            # The first matmul of the next pass depends on all 16 copies of
            # the previous pass (true all-to-all), which would give its
            # LDWEIGHTS 2+ sync waits - the encoding holds only one.  Wait
            # dedup in Tile only works matmul-to-matmul, so interpose a tiny
            # dummy matmul (M=1, N=1) that carries the DVE-side wait; the
            # first real matmul then only needs the ACT-side wait.
            mmi = nc.tensor.matmul(
                out=psA[0:32, 0:NB],
                lhsT=btile[:, 0:32],
                rhs=btile[:, 0:NB],
                start=True,
                stop=True,
            )
            for cp in dve_copies:
                add_dep_helper(
                    mmi.ins, cp.ins, sync=True, reason="pass boundary wait split"
                )

        d1 = conv_pass(v0, v1, 0, chunk1, dst1, 0)
        pass_boundary(d1, 0)
        d2 = conv_pass(v1, v2, 1, chunk2, dst2, 1)
        pass_boundary(d2, 1)
        conv_pass(v2, v3, 2, chunk3, dst3, 2)

    nc.compile()
    _cache[mode] = nc
    return nc


def _prep_inputs(x, sigma_x, sigma_y, sigma_z, mode):
    _, np_dt, NB, _ = _mode_params(mode)
    gx = _gauss1d(float(sigma_x), SPACING[0])
    gy = _gauss1d(float(sigma_y), SPACING[1])
    gz = _gauss1d(float(sigma_z), SPACING[2])
    bmats = np.concatenate(
        [_banded(gx, NB), _banded(gy, NB), _banded(gz, NB)], axis=1
    ).astype(np_dt)
    x = np.asarray(x, dtype=np.float32).reshape(N_CORES, P, HW)
    in_maps = [
        {"x": np.ascontiguousarray(x[i]).astype(np_dt), "bmats": bmats}
        for i in range(N_CORES)
    ]
    return in_maps


def _run(x, sigma_x, sigma_y, sigma_z, mode=None, trace=False):
    from concourse.bass_utils import run_bass_kernel_spmd

    mode = mode or MODE
    nc = _build(mode)
    in_maps = _prep_inputs(x, sigma_x, sigma_y, sigma_z, mode)
    res = run_bass_kernel_spmd(nc, in_maps, core_ids=list(range(N_CORES)), trace=trace)
    y = np.stack([np.asarray(res.results[i]["y"]) for i in range(N_CORES)])
    y = y.reshape(N_CORES, 1, P, P, P).astype(np.float32)
    return y, res


def kernel(x, sigma_x, sigma_y, sigma_z):
    y, _ = _run(x, sigma_x, sigma_y, sigma_z)
    return y



# revision 4
# speedup vs baseline: 1.6111x; 1.6111x over previous
"""Trainium2 Bass kernel for nn_Blurring_Model: 3D Gaussian blur (9^3 PSF)
on x[8,1,128,128,128] fp32, data-parallel over batch across 8 NeuronCores.

Method (per core, volume V[128,128,128]):
  The 3D PSF is separable: three 9-tap 1D convolutions along D, H, W.
  Each 1D conv along the SBUF partition axis is a matmul with a banded
  128x128 matrix B (B[d, d'] = g[d - d' + 4]); zero band entries handle
  the 'same' zero padding exactly.

  Every pass uses the volume chunk as the matmul's stationary operand
  (lhsT) and streams B, so out = chunk^T @ B both convolves the current
  partition axis AND rotates the next axis onto partitions ("rotation
  matmul") - no explicit transposes:

    V0 [D,(Hmaj,Wmin)] --pass1 conv D (chunks: fixed w)--> V1 [H,(Dmaj,Wmin)]
    V1                 --pass2 conv H (chunks: fixed d)--> V2 [W,(Hmaj,Dmin)]
    V2                 --pass3 conv W (chunks: fixed h)--> V3 [D,(Hmaj,Wmin)]

  128 matmuls of [K=128, M=128] x [K=128, N] per pass.  In "f32r" mode the
  streamed B is padded to N=256 so the PE runs float32r at 1 cycle/row.
  PSUM->SBUF copies are batched 8 chunks at a time and alternate between
  the Vector and Scalar engines.
"""

import sys

if "/opt/trn_rl_repo" not in sys.path:
    sys.path.insert(0, "/opt/trn_rl_repo")

import numpy as np

KERNEL_SIZE = 9
SPACING = (4.0, 4.0, 4.0)
CENTER = (KERNEL_SIZE - 1) / 2.0
PAD = (KERNEL_SIZE - 1) // 2
P = 128
HW = P * P
N_CORES = 8

# "f32r" | "f32" | "fp16" | "bf16"
MODE = "f32r"

GRP = 8           # matmul chunks per PSUM group / copy
NGRP = P // GRP   # groups per pass

_cache = {}


def _gauss1d(sigma, spacing):
    s = float(sigma) / spacing
    xs = np.arange(KERNEL_SIZE, dtype=np.float64)
    g = np.exp(-((xs - CENTER) ** 2) / (2.0 * s * s))
    g = g / g.sum()
    return g.astype(np.float32)


def _banded(g, ncols):
    # B[d, d'] = g[d - d' + PAD] for |d - d'| <= PAD, else 0.
    B = np.zeros((P, ncols), dtype=np.float32)
    d = np.arange(P)
    for i in range(KERNEL_SIZE):
        off = i - PAD
        dp = d - off
        m = (dp >= 0) & (dp < P)
        B[d[m], dp[m]] = g[i]
    return B


def _mode_params(mode):
    import concourse.mybir as mybir

    f32 = mybir.dt.float32
    if mode == "fp16":
        return mybir.dt.float16, np.float16, 128, 96
    if mode == "bf16":
        import ml_dtypes

        return mybir.dt.bfloat16, np.dtype(ml_dtypes.bfloat16), 128, 96
    if mode == "f32":
        return f32, np.float32, 128, 96
    if mode == "f32r":
        return f32, np.float32, 256, 120
    if mode in ("a16", "b16"):
        return mybir.dt.float16, np.float16, 128, 12
    raise ValueError(mode)


def _build(mode):
    """Builds the SPMD Bass module (single program, run on 8 cores)."""
    if mode in _cache:
        return _cache[mode]

    from contextlib import ExitStack

    import concourse.bacc as bacc
    import concourse.bass as bass
    import concourse.mybir as mybir
    import concourse.tile as tile

    f32 = mybir.dt.float32
    dt_dat, _, NB, n_warm = _mode_params(mode)
    f32r_mode = mode == "f32r"
    # dtype the matmuls consume (and the volume tiles are stored as)
    dt_vol = mybir.dt.float32r if f32r_mode else dt_dat

    nc = bacc.Bacc(trn_type="TRN2", target_bir_lowering=False, debug=False)
    x_in = nc.declare_dram_parameter(
        "x", [P, HW], dt_vol if f32r_mode else dt_dat, isOutput=False
    ).ap()
    b_in = nc.declare_dram_parameter("bmats", [P, 3 * NB], dt_dat, isOutput=False).ap()
    y_out = nc.declare_dram_parameter("y", [P, HW], f32, isOutput=True).ap()

    with ExitStack() as ctx:
        tc = ctx.enter_context(tile.TileContext(nc))
        vol = ctx.enter_context(tc.tile_pool(name="vol", bufs=3))
        consts = ctx.enter_context(tc.tile_pool(name="consts", bufs=1))
        pspool = ctx.enter_context(tc.tile_pool(name="ps", bufs=2, space="PSUM"))

        braw = consts.tile([P, 3 * NB], dt_dat, name="braw", tag="braw")
        nc.sync.dma_start(out=braw[:], in_=b_in[:])
        btile = consts.tile([P, 3 * NB], dt_vol, name="btile", tag="b")
        # engine copy rounds f32 -> f32r as the BIR verifier requires
        nc.vector.tensor_copy(out=btile[:], in_=braw[:])
        scratch = consts.tile([P, 128], f32, name="scratch", tag="scratch")

        v0 = vol.tile([P, HW], dt_vol, name="v0", tag="vol")
        # split across both HWDGE rings (SP + ACT) for DMA parallelism
        nc.sync.dma_start(out=v0[:, 0 : HW // 2], in_=x_in[:, 0 : HW // 2])
        nc.scalar.dma_start(out=v0[:, HW // 2 :], in_=x_in[:, HW // 2 :])

        # Two persistent PSUM tiles, ping-ponged by group parity.  Never
        # recycling tiles keeps the PE->PE PSUM WAW deps semaphore-free
        # (program order), so each matmul carries at most ONE sync wait -
        # the LDWEIGHTS instruction encoding cannot hold more.
        psA = pspool.tile([P, GRP * NB], f32, name="psA", tag="ps")
        psB = pspool.tile([P, GRP * NB], f32, name="psB", tag="ps")

        # Warm the ACT tables (Copy) and the PE HAM clock gate while the
        # 8MB input DMA is in flight.
        nc.scalar.copy(out=scratch[:], in_=braw[:, 0:128])
        for _ in range(n_warm):
            nc.tensor.matmul(
                out=psA[:, 0:NB],
                lhsT=btile[:, 0:128],
                rhs=btile[:, 0:NB],
                start=True,
                stop=True,
            )

        v1 = vol.tile([P, HW], dt_vol, name="v1", tag="vol")
        v2 = vol.tile([P, HW], dt_vol, name="v2", tag="vol")
        v3 = vol.tile([P, HW], f32, name="v3", tag="vol")

        def conv_pass(src, dst, b_idx, chunk_fn, dst_ap_fn, pass_idx):
            b_ap = btile[:, b_idx * NB : (b_idx + 1) * NB]
            dve_copies = []
            for g in range(NGRP):
                ps = psA if g % 2 == 0 else psB
                for c in range(GRP):
                    nc.tensor.matmul(
                        out=ps[:, c * NB : (c + 1) * NB],
                        lhsT=chunk_fn(src, g * GRP + c),
                        rhs=b_ap,
                        start=True,
                        stop=True,
                    )
                src_ap, dst_ap = dst_ap_fn(ps, dst, g)
                if g % 2 == 0:
                    cp = nc.vector.tensor_copy(out=dst_ap, in_=src_ap)
                    dve_copies.append(cp)
                else:
                    nc.scalar.copy(out=dst_ap, in_=src_ap)
                if pass_idx == 2:
                    dma_eng = nc.sync if g % 2 == 0 else nc.scalar
                    dma_eng.dma_start(
                        out=y_out[:, g * 1024 : (g + 1) * 1024],
                        in_=v3[:, g * 1024 : (g + 1) * 1024],
                    )
            return dve_copies

        # All passes: chunk = strided lhsT read (fixed minor index of the
        # free dim), copy dst = CONTIGUOUS 1024-elem block (so Tile's
        # subtile dep tracking sees disjoint copy writes - no spurious
        # cross-engine WAW waits).
        #
        # pass 1: conv D.  V0[d, h*128+w]; chunk w -> [d, h];
        # out [h, d'] -> V1[h, w*128 + d']
        def chunk1(src, w):
            return src.rearrange("p (h w) -> p w h", w=P)[:, w, :]

        # pass 2: conv H.  V1[h, w*128+d]; chunk d -> [h, w];
        # out [w, h'] -> V2[w, d*128 + h']
        def chunk2(src, d):
            return src.rearrange("p (w d) -> p d w", d=P)[:, d, :]

        # pass 3: conv W.  V2[w, d*128+h]; chunk h -> [w, d];
        # out [d, w'] -> V3[d, h*128 + w']
        def chunk3(src, h):
            return src.rearrange("p (d h) -> p h d", h=P)[:, h, :]

        def dst_block(ps, dst, g):
            src_ap = ps.rearrange("p (c n) -> p c n", n=NB)[:, :, 0:128]
            dst_ap = dst.rearrange("p (c n) -> p c n", n=128)[:, g * GRP : (g + 1) * GRP, :]
            return src_ap, dst_ap

        dst1 = dst2 = dst3 = dst_block

        from concourse.tile_rust import add_dep_helper

        def pass_boundary(dve_copies, idx):
            # The first matmul of the next pass depends on all 16 copies of
            # the previous pass (true all-to-all), which would give its
            # LDWEIGHTS 2+ sync waits - the encoding holds only one.  Wait
            # dedup in Tile only works matmul-to-matmul, so interpose a tiny
            # dummy matmul (M=1, N=1) that carries the DVE-side wait; the
            # first real matmul then only needs the ACT-side wait.
            mmi = nc.tensor.matmul(
                out=psA[0:32, 0:NB],
                lhsT=btile[:, 0:32],
                rhs=btile[:, 0:NB],
                start=True,
                stop=True,
            )
            for cp in dve_copies:
                add_dep_helper(
                    mmi.ins, cp.ins, sync=True, reason="pass boundary wait split"
                )

        d1 = conv_pass(v0, v1, 0, chunk1, dst1, 0)
        pass_boundary(d1, 0)
        d2 = conv_pass(v1, v2, 1, chunk2, dst2, 1)
        pass_boundary(d2, 1)
        conv_pass(v2, v3, 2, chunk3, dst3, 2)

    nc.compile()
    _cache[mode] = nc
    return nc


def _build_v2(mode):
    """fp16 pipeline, chunk-major pass 1 (contiguous lhsT + input-DMA
    overlap), fp16 output.  mode "b16" additionally scatters the pass-1/2
    PSUM->SBUF copies so passes 2 and 3 also read contiguous lhsT blocks
    (FWL-eligible weight loads on all passes).

    Axis bookkeeping (volume axes d,h,w; primes = convolved):
      V0 [d, h*128+w]   pass1: chunk h, lhsT=[d,w] contig, rhs=Bd -> out [w, d']
      a16: V1[w, h*128+d']  (contig copy)   pass2 chunk d' (strided lhsT)
      b16: V1[w, d'*128+h]  (scatter copy)  pass2 chunk d' (contig lhsT)
      pass2: contract w, rhs=Bw -> out [h, w']
      a16: V2[h, d'*128+w'] (contig)        pass3 chunk w' (strided)
      b16: V2[h, w'*128+d'] (scatter)       pass3 chunk w' (contig)
      pass3: contract h, rhs=Bh -> out [d', h'] -> V3[d', w'*128+h'] contig
      => DRAM y layout [d][w][h]; host transposes back to [d][h][w].
    """
    if mode in _cache:
        return _cache[mode]

    from contextlib import ExitStack

    import concourse.bacc as bacc
    import concourse.mybir as mybir
    import concourse.tile as tile

    f16 = mybir.dt.float16
    f32 = mybir.dt.float32
    NB = 128
    scatter = mode == "b16"
    GRP2 = 16          # chunks per PSUM group
    NGRP2 = P // GRP2  # 8 groups per pass
    GCOL = GRP2 * NB   # columns per group (2048)
    N_WARM = 12

    nc = bacc.Bacc(trn_type="TRN2", target_bir_lowering=False, debug=False)
    x_in = nc.declare_dram_parameter("x", [P, HW], f16, isOutput=False).ap()
    b_in = nc.declare_dram_parameter("bmats", [P, 3 * NB], f16, isOutput=False).ap()
    y_out = nc.declare_dram_parameter("y", [P, HW], f16, isOutput=True).ap()

    with ExitStack() as ctx:
        tc = ctx.enter_context(tile.TileContext(nc))
        vol = ctx.enter_context(tc.tile_pool(name="vol", bufs=3))
        consts = ctx.enter_context(tc.tile_pool(name="consts", bufs=1))
        pspool = ctx.enter_context(tc.tile_pool(name="ps", bufs=2, space="PSUM"))

        btile = consts.tile([P, 3 * NB], f16, name="btile", tag="b")
        nc.sync.dma_start(out=btile[:], in_=b_in[:])
        scratch = consts.tile([P, 128], f32, name="scratch", tag="scratch")

        v0 = vol.tile([P, HW], f16, name="v0", tag="vol")
        # input DMA in 8 pieces of 2048 cols (512KB) so pass-1 groups can
        # start as soon as their piece lands; alternate the two HWDGE rings
        for pc in range(NGRP2):
            eng = nc.sync if pc % 2 == 0 else nc.scalar
            eng.dma_start(
                out=v0[:, pc * GCOL : (pc + 1) * GCOL],
                in_=x_in[:, pc * GCOL : (pc + 1) * GCOL],
            )

        psA = pspool.tile([P, GCOL], f32, name="psA", tag="ps")
        psB = pspool.tile([P, GCOL], f32, name="psB", tag="ps")

        # warm ACT tables + PE while the first input piece is in flight
        nc.scalar.copy(out=scratch[:], in_=btile[:, 0:128])
        for _ in range(N_WARM):
            nc.tensor.matmul(
                out=psA[:, 0:NB],
                lhsT=btile[:, 0:128],
                rhs=btile[:, 0:NB],
                start=True,
                stop=True,
            )

        v1 = vol.tile([P, HW], f16, name="v1", tag="vol")
        v2 = vol.tile([P, HW], f16, name="v2", tag="vol")
        v3 = vol.tile([P, HW], f16, name="v3", tag="vol")

        from concourse.tile_rust import add_dep_helper

        def conv_pass(src, dst, b_idx, lhsT_fn, scatter_dst, pass_idx):
            b_ap = btile[:, b_idx * NB : (b_idx + 1) * NB]
            dve_copies = []
            for g in range(NGRP2):
                ps = psA if g % 2 == 0 else psB
                for c in range(GRP2):
                    nc.tensor.matmul(
                        out=ps[:, c * NB : (c + 1) * NB],
                        lhsT=lhsT_fn(src, g * GRP2 + c),
                        rhs=b_ap,
                        start=True,
                        stop=True,
                    )
                if scatter_dst:
                    # dst col = n*128 + chunk; DVE takes n<64, ACT n>=64 so
                    # each engine's write RANGE stays disjoint (no spurious
                    # cross-engine WAW from Tile's range tracker)
                    src3 = ps.rearrange("p (c n) -> p c n", n=NB)
                    dst3 = dst.rearrange("p (n c) -> p c n", c=P)[
                        :, g * GRP2 : (g + 1) * GRP2, :
                    ]
                    cp = nc.vector.tensor_copy(
                        out=dst3[:, :, 0:64], in_=src3[:, :, 0:64]
                    )
                    dve_copies.append(cp)
                    nc.scalar.copy(out=dst3[:, :, 64:NB], in_=src3[:, :, 64:NB])
                else:
                    src_ap = ps[:, 0:GCOL]
                    dst_ap = dst[:, g * GCOL : (g + 1) * GCOL]
                    if g % 2 == 0:
                        cp = nc.vector.tensor_copy(out=dst_ap, in_=src_ap)
                        dve_copies.append(cp)
                    else:
                        nc.scalar.copy(out=dst_ap, in_=src_ap)
                if pass_idx == 2:
                    dma_eng = nc.sync if g % 2 == 0 else nc.scalar
                    dma_eng.dma_start(
                        out=y_out[:, g * GCOL : (g + 1) * GCOL],
                        in_=v3[:, g * GCOL : (g + 1) * GCOL],
                    )
            return dve_copies

        # pass 1: chunk h -> contiguous lhsT [d, w]
        def lhsT1(src, h):
            return src[:, h * 128 : (h + 1) * 128]

        if scatter:
            # V1[w, d'*128+h]: chunk d' contiguous
            def lhsT2(src, d):
                return src[:, d * 128 : (d + 1) * 128]

            # V2[h, w'*128+d']: chunk w' contiguous
            def lhsT3(src, w):
                return src[:, w * 128 : (w + 1) * 128]
        else:
            # V1[w, h*128+d']: chunk d' strided
            def lhsT2(src, d):
                return src.rearrange("p (h d) -> p d h", d=P)[:, d, :]

            # V2[h, d'*128+w']: chunk w' strided
            def lhsT3(src, w):
                return src.rearrange("p (d w) -> p w d", w=P)[:, w, :]

        def pass_boundary(dve_copies):
            mmi = nc.tensor.matmul(
                out=psA[0:32, 0:NB],
                lhsT=btile[:, 0:32],
                rhs=btile[:, 0:NB],
                start=True,
                stop=True,
            )
            for cp in dve_copies:
                add_dep_helper(
                    mmi.ins, cp.ins, sync=True, reason="pass boundary wait split"
                )

        d1 = conv_pass(v0, v1, 0, lhsT1, scatter, 0)
        pass_boundary(d1)
        d2 = conv_pass(v1, v2, 2, lhsT2, scatter, 1)
        pass_boundary(d2)
        conv_pass(v2, v3, 1, lhsT3, False, 2)

    nc.compile()
    _cache[mode] = nc
    return nc


def _prep_inputs(x, sigma_x, sigma_y, sigma_z, mode):
    _, np_dt, NB, _ = _mode_params(mode)
    gx = _gauss1d(float(sigma_x), SPACING[0])
    gy = _gauss1d(float(sigma_y), SPACING[1])
    gz = _gauss1d(float(sigma_z), SPACING[2])
    bmats = np.concatenate(
        [_banded(gx, NB), _banded(gy, NB), _banded(gz, NB)], axis=1
    ).astype(np_dt)
    x = np.asarray(x, dtype=np.float32).reshape(N_CORES, P, HW)
    in_maps = [
        {"x": np.ascontiguousarray(x[i]).astype(np_dt), "bmats": bmats}
        for i in range(N_CORES)
    ]
    return in_maps


def _run(x, sigma_x, sigma_y, sigma_z, mode=None, trace=False):
    from concourse.bass_utils import run_bass_kernel_spmd

    mode = mode or MODE
    v2 = mode in ("a16", "b16")
    nc = _build_v2(mode) if v2 else _build(mode)
    in_maps = _prep_inputs(x, sigma_x, sigma_y, sigma_z, mode)
    res = run_bass_kernel_spmd(nc, in_maps, core_ids=list(range(N_CORES)), trace=trace)
    y = np.stack([np.asarray(res.results[i]["y"]) for i in range(N_CORES)])
    y = y.reshape(N_CORES, 1, P, P, P).astype(np.float32)
    if v2:
        # device produced [d][w][h]; back to [d][h][w]
        y = np.ascontiguousarray(y.transpose(0, 1, 2, 4, 3))
    return y, res


def kernel(x, sigma_x, sigma_y, sigma_z):
    y, _ = _run(x, sigma_x, sigma_y, sigma_z)
    return y

